# revision 17
# baseline (speedup 1.0000x reference)
"""MoE ExpertLayer kernel for Trainium2 (8 NeuronCores, data-parallel over tokens).

Reference computation (B=4, S=2048, D=1024, E=8):
    logits  = x @ W_router.T + b_router          # [B,S,E]
    probs   = softmax(logits, axis=-1)
    y_e     = x @ W_experts[e].T + b_experts[e]  # all experts, dense
    out     = sum_e probs[..., e] * y_e          # [B,S,D]

Sharding: data-parallel over the flattened token axis (8192 tokens -> 1024
tokens per core). Every core receives the full (transposed) expert weights and
computes its token shard end-to-end; no collectives are needed.

Per-core dataflow. The kernel is PE-bound: 1024 expert matmuls + 16 router +
16 bias-fold matmuls at the hardware's 216ns/MM N=512 bf16 cadence is ~228us
of tensor-engine time; everything else hides behind it. The trace-driven
structure:
  - All operands host-cast to bf16, pre-tiled so the contraction dim d sits on
    SBUF partitions with contiguous per-partition DMA chunks.
  - Engine bring-up costs a fixed ~6.6us before the first DMA trigger can run;
    the first data lands ~8.3us. A short burst of dummy matmuls on a zeroed
    tile starts at ~6.7us so the PE clock ramp (which otherwise runs the first
    ~20 matmuls at half cadence) burns during the DMA-dead window.
  - Startup DMA: sync ring carries xT th0 (4 chunks) then th1; scalar ring
    carries the tiny router/bias tensors then ALL of expert 0's weights in 4
    chunks — the two rings fill x and w0 in parallel at ~300GB/s each, and the
    PE chases chunks: router th0, expert0 tt0, router th1, expert0 tt1..7.
  - Router: W_router.T stationary -> logitsT [8, 512] PSUM per token half;
    b_router added on the ACT copy out of PSUM; each [8, 128] slice transposed
    back on the PE and soft-maxed token-major with [128, 1] DVE/ACT ops.
  - Bias fold: sum_e probs[t,e]*b_e[f] is a K=8 matmul with probs.T
    stationary, copied into the SBUF accumulator before the expert loop.
  - Experts: per (token tile, f-half), PSUM accumulates 8 d-tile matmuls (the
    two f-halves share each stationary load); the combine
    acc = psum * probs[:,e] + acc is one fused DVE scalar_tensor_tensor op.
  - The final expert's combines write bf16 tiles that stream straight out to
    DRAM (half the store bytes of f32; the host upcasts), and the last tile
    goes out as two quarter-tiles on both rings to shorten the drain tail.
"""

import os
import sys

for _p in ("/opt/trn_rl_repo", "/root/.axon_site/_ro/trn_rl_repo"):
    if os.path.isdir(_p) and _p not in sys.path:
        sys.path.insert(0, _p)

from contextlib import ExitStack

import ml_dtypes
import numpy as np

import concourse.bass as bass
import concourse.mybir as mybir
import concourse.tile as tile
from concourse import bacc
from concourse.bass import ts
from concourse.bass_utils import run_bass_kernel_spmd
from concourse.masks import make_identity

B, S, D, E = 4, 2048, 1024, 8
N_CORES = 8
T = B * S // N_CORES  # tokens per core = 1024
P = 128               # partitions
TT = T // P           # token tiles per core = 8
DT = D // P           # contraction tiles = 8
FN = 512              # matmul moving free dim (one PSUM bank of fp32)
FH = D // FN          # output column halves = 2

MODE = os.environ.get("KERNEL_MODE", "bf16")  # bf16 | f32r | f32


def _compute_dt(mode):
    return {
        "bf16": mybir.dt.bfloat16,
        "f32r": mybir.dt.float32r,
        "f32": mybir.dt.float32,
    }[mode]


def _np_dt(mode):
    return {"bf16": ml_dtypes.bfloat16, "f32r": np.float32, "f32": np.float32}[mode]


def build(mode=MODE):
    """Build the per-core Bass/Tile program (identical SPMD program on all cores)."""
    cdt = _compute_dt(mode)
    f32 = mybir.dt.float32

    nc = bacc.Bacc("TRN2", target_bir_lowering=False, debug=False)

    # Inputs are pre-tiled on the host to [partition, ..., d-tile, ...] so
    # every DMA reads long contiguous per-partition chunks — the naive
    # [D, ...] layout yields 2KB strided descriptors that throttle a HWDGE
    # queue. xT is additionally split by token half so the router can start
    # as soon as the first half lands.
    TH = 2          # token halves per core
    THT = T // TH   # 512 tokens per half
    xT_d = nc.dram_tensor("xT", [P, TH, DT, THT], cdt, kind="ExternalInput").ap()
    Wt_d = nc.dram_tensor("Wt", [E, P, DT, D], cdt, kind="ExternalInput").ap()
    be_d = nc.dram_tensor("be", [E, D], cdt, kind="ExternalInput").ap()
    WrT_d = nc.dram_tensor("WrT", [P, DT, E], cdt, kind="ExternalInput").ap()
    brT_d = nc.dram_tensor("brT", [E, 1], f32, kind="ExternalInput").ap()
    out_d = nc.dram_tensor("out", [T, D], cdt, kind="ExternalOutput").ap()

    with tile.TileContext(nc) as tc, ExitStack() as ctx:
        singles = ctx.enter_context(tc.tile_pool(name="singles", bufs=1))
        wpool = ctx.enter_context(tc.tile_pool(name="wpool", bufs=4))
        small = ctx.enter_context(tc.tile_pool(name="small", bufs=4))
        opool = ctx.enter_context(tc.tile_pool(name="opool", bufs=4))
        ppool = ctx.enter_context(tc.tile_pool(name="psum_e", bufs=2, space="PSUM"))
        pbias = ctx.enter_context(tc.tile_pool(name="psum_b", bufs=1, space="PSUM"))
        prout = ctx.enter_context(tc.tile_pool(name="psum_r", bufs=1, space="PSUM"))

        # Two HWDGE rings (sync=SP, scalar=ACT); each sustains ~300GB/s on
        # these contiguous-per-partition patterns, but a trigger instruction
        # costs ~0.6us on the issuing engine, so chunks stay >=256KB.
        hwdge = [nc.sync, nc.scalar]

        identf = singles.tile([P, P], f32)
        make_identity(nc, identf)

        # Resident tensors. The sync ring starts delivering at ~8.6us and
        # ramps quickly; the scalar ring crawls (~40GB/s) until ~15us. So
        # everything the first ~16us of compute needs rides sync in
        # consumption order (WrT -> th0 -> w0 d0..3), and scalar carries the
        # later-needed pieces (th1 for the second router half, w0 d4..7).
        WrT = singles.tile([P, DT, E], cdt)
        nc.sync.dma_start(out=WrT, in_=WrT_d)
        be = singles.tile([E, D], cdt)
        nc.scalar.dma_start(out=be, in_=be_d)
        brT = singles.tile([E, 1], f32)
        nc.scalar.dma_start(out=brT, in_=brT_d)

        xT = singles.tile([P, TH, DT, THT], cdt)
        w0 = wpool.tile([P, DT, D], cdt, tag="w")
        hd = DT // 2
        for c in range(0, DT, 2):
            nc.sync.dma_start(out=xT[:, 0, c : c + 2], in_=xT_d[:, 0, c : c + 2])
        nc.sync.dma_start(out=w0[:, 0:2], in_=Wt_d[0, :, 0:2])
        nc.sync.dma_start(out=w0[:, 2:4], in_=Wt_d[0, :, 2:4])
        nc.scalar.dma_start(out=w0[:, 4:6], in_=Wt_d[0, :, 4:6])
        nc.scalar.dma_start(out=w0[:, 6:8], in_=Wt_d[0, :, 6:8])
        nc.scalar.dma_start(out=xT[:, 1], in_=xT_d[:, 1])

        acc = singles.tile([P, TT, D], f32)
        probs = singles.tile([P, TT, E], f32)
        probsT = singles.tile([E, TT, P], cdt)
        logitsT = singles.tile([E, TT, P], f32)

        # ---- Router ----
        # logitsT[e, t] accumulates in PSUM with W_router as the stationary
        # (16 N=512 matmuls instead of 64 N=8 ones); b_router is added on the
        # ACT copy out of PSUM (per-partition bias). Each token tile is then
        # transposed back to [tok, e] on the PE and soft-maxed with cheap
        # [128, 1] per-token reductions; the bias-fold matmuls interleave
        # per-tt so the PE has useful work while the other xT half lands.
        out_dst = out_d.rearrange("(tt p) f -> p tt f", p=P)

        TQ = TT // TH  # token tiles per half = 4

        def router_mms(th):
            t4 = slice(th * TQ, (th + 1) * TQ)
            pr = prout.tile([E, THT], f32, tag="pr")
            for dt_ in range(DT):
                nc.tensor.matmul(
                    pr, WrT[:, dt_, :], xT[:, th, dt_, :],
                    start=(dt_ == 0), stop=(dt_ == DT - 1),
                )
            nc.scalar.activation(
                out=logitsT[:, t4, :].rearrange("e a b -> e (a b)"), in_=pr,
                func=mybir.ActivationFunctionType.Identity, bias=brT, scale=1.0,
            )

        def sm_chains(th):
            # all four logit transposes back-to-back into one grouped PSUM
            # tile, then the per-tile DVE/ACT softmax chains drain off the
            # PE's critical path
            pTq = prout.tile([P, TQ, E], f32, tag="pTq")
            for j in range(TQ):
                tt = th * TQ + j
                nc.tensor.transpose(pTq[:, j, :], logitsT[:, tt, :], identf[:E, :E])
            for j in range(TQ):
                tt = th * TQ + j
                negmax = small.tile([P, 1], f32, tag="negmax")
                nc.vector.reduce_max(
                    out=negmax, in_=pTq[:, j, :], axis=mybir.AxisListType.X,
                    negate=True,
                )
                z = small.tile([P, E], f32, tag="z")
                ssum = small.tile([P, 1], f32, tag="ssum")
                nc.scalar.activation(
                    out=z, in_=pTq[:, j, :],
                    func=mybir.ActivationFunctionType.Exp,
                    bias=negmax, scale=1.0, accum_out=ssum,
                )
                rec = small.tile([P, 1], f32, tag="rec")
                nc.vector.reciprocal(rec, ssum)
                nc.vector.tensor_scalar_mul(probs[:, tt, :], z, rec)

        def sm_bias(tt):
            # probs.T (bf16) plus the bias fold for one token tile:
            # acc[t, f] = sum_e probs[t, e] * b_experts[e, f]
            # (PSUM->SBUF copies on ACT, not DVE — on DVE they backlog the
            # expert combines behind the softmax chain, which stalls PSUM
            # recycling and with it the matmul stream)
            pT2 = prout.tile([E, P], f32, tag="pT2")
            nc.tensor.transpose(pT2, probs[:, tt, :], identf)
            nc.vector.tensor_copy(probsT[:, tt, :], pT2)
            for fh in range(FH):
                pb = pbias.tile([P, FN], f32, tag="pb")
                nc.tensor.matmul(
                    pb, probsT[:, tt, :], be[:, ts(fh, FN)],
                    start=True, stop=True,
                )
                nc.scalar.activation(
                    out=acc[:, tt, ts(fh, FN)], in_=pb,
                    func=mybir.ActivationFunctionType.Identity,
                )

        def expert_tt_mms(w, tt, pe0, pe1, dts, start, stop):
            # one stationary load serves both output halves: accumulate
            # the fh=0 and fh=1 PSUM groups side by side per d-tile
            TH_ = TT // TH
            for j, dt_ in enumerate(dts):
                lhsT = xT[:, tt // TH_, dt_, ts(tt % TH_, P)]
                st = start and j == 0
                sp = stop and j == len(dts) - 1
                nc.tensor.matmul(pe0, lhsT, w[:, dt_, 0:FN], start=st, stop=sp)
                nc.tensor.matmul(
                    pe1, lhsT, w[:, dt_, FN : 2 * FN], start=st, stop=sp
                )

        def combine(e, tt, pe0, pe1):
            for fh, pe_ in ((0, pe0), (1, pe1)):
                    if e < E - 1:
                        # acc = psum * probs[:, e] + acc  (one fused DVE op)
                        nc.vector.scalar_tensor_tensor(
                            out=acc[:, tt, ts(fh, FN)],
                            in0=pe_,
                            scalar=probs[:, tt, e : e + 1],
                            in1=acc[:, tt, ts(fh, FN)],
                            op0=mybir.AluOpType.mult,
                            op1=mybir.AluOpType.add,
                        )
                    elif tt == TT - 1 and fh == FH - 1:
                        # very last half-tile: two quarter combines + stores
                        # on both rings to shorten the final drain
                        ob = opool.tile([P, FN], cdt, tag=f"ob{fh}")
                        for q in range(2):
                            qs = ts(q, FN // 2)
                            nc.vector.scalar_tensor_tensor(
                                out=ob[:, qs],
                                in0=pe_[:, qs],
                                scalar=probs[:, tt, e : e + 1],
                                in1=acc[:, tt, fh * FN + q * (FN // 2) :][
                                    :, : FN // 2
                                ],
                                op0=mybir.AluOpType.mult,
                                op1=mybir.AluOpType.add,
                            )
                            hwdge[q].dma_start(
                                out=out_dst[:, tt, fh * FN + q * (FN // 2) :][
                                    :, : FN // 2
                                ],
                                in_=ob[:, qs],
                            )
                    else:
                        # final expert: combine straight into a bf16 tile and
                        # stream it out now so stores overlap remaining compute
                        ob = opool.tile([P, FN], cdt, tag=f"ob{fh}")
                        nc.vector.scalar_tensor_tensor(
                            out=ob,
                            in0=pe_,
                            scalar=probs[:, tt, e : e + 1],
                            in1=acc[:, tt, ts(fh, FN)],
                            op0=mybir.AluOpType.mult,
                            op1=mybir.AluOpType.add,
                        )
                        hwdge[fh].dma_start(
                            out=out_dst[:, tt, ts(fh, FN)], in_=ob
                        )

        def expert_block(e, w, tts, dts=tuple(range(DT))):
            for tt in tts:
                pe0 = ppool.tile([P, FN], f32, tag="pe0")
                pe1 = ppool.tile([P, FN], f32, tag="pe1")
                expert_tt_mms(w, tt, pe0, pe1, dts, True, True)
                combine(e, tt, pe0, pe1)

        # ---- Ramp: router th0 chases the th0 chunks during the early-DMA
        # crawl (softmax-chain latencies hide in the starvation); expert 0's
        # d-loop follows w0's two-ring arrival order; router th1 runs once
        # th1 lands, and its four probs.T/bias-fold steps each hide behind a
        # full expert-0 matmul block so the chain latency never stalls the
        # PE ----
        E0_DTS = (0, 1, 4, 5, 2, 3, 6, 7)
        router_mms(0)
        sm_chains(0)
        for tt in range(0, TQ):
            sm_bias(tt)
        expert_block(0, w0, range(0, TQ), dts=E0_DTS)
        router_mms(1)
        sm_chains(1)
        for tt in range(TQ, TT):
            pe0 = ppool.tile([P, FN], f32, tag="pe0")
            pe1 = ppool.tile([P, FN], f32, tag="pe1")
            expert_tt_mms(w0, tt, pe0, pe1, E0_DTS, True, True)
            sm_bias(tt)
            combine(0, tt, pe0, pe1)

        # ---- Steady state: stream experts 1..7 across both HWDGE rings ----
        half = DT // 2
        for e in range(1, E):
            w = wpool.tile([P, DT, D], cdt, tag="w")
            nc.sync.dma_start(out=w[:, :half, :], in_=Wt_d[e, :, :half, :])
            nc.scalar.dma_start(out=w[:, half:, :], in_=Wt_d[e, :, half:, :])
            expert_block(e, w, range(TT))

    nc.compile()
    return nc


def prep_inputs(x, W_experts, b_experts, W_router, b_router, mode=MODE):
    """Host-side marshalling: shard tokens, transpose so the contraction dim
    is DMA-contiguous onto SBUF partitions, cast to the compute dtype."""
    ndt = _np_dt(mode)
    x = np.asarray(x, dtype=np.float32).reshape(B * S, D)
    # [E, D_out, D_in] -> transposed + tiled to [E, P, DT, D_out] so each SBUF
    # partition reads one contiguous 16KB chunk per DMA
    Wt = np.ascontiguousarray(
        np.asarray(W_experts, dtype=np.float32)
        .transpose(0, 2, 1)            # [E, D_in, D_out]
        .reshape(E, DT, P, D)
        .transpose(0, 2, 1, 3)         # [E, P, DT, D_out]
    ).astype(ndt)
    WrT = np.ascontiguousarray(
        np.asarray(W_router, dtype=np.float32)
        .T.reshape(DT, P, E)
        .transpose(1, 0, 2)            # [P, DT, E]
    ).astype(ndt)
    be = np.asarray(b_experts, dtype=np.float32).astype(ndt)
    brT = np.asarray(b_router, dtype=np.float32).reshape(E, 1)
    TH, THT = 2, T // 2
    in_maps = []
    for c in range(N_CORES):
        xT = np.ascontiguousarray(
            x[c * T : (c + 1) * T, :]
            .T.reshape(DT, P, TH, THT)
            .transpose(1, 2, 0, 3)     # [P, TH, DT, THT]
        ).astype(ndt)
        in_maps.append({"xT": xT, "Wt": Wt, "be": be, "WrT": WrT, "brT": brT})
    return in_maps


_BUILT = {}


def get_built(mode=MODE):
    if mode not in _BUILT:
        _BUILT[mode] = build(mode)
    return _BUILT[mode]


def wait_device_ready(max_tries=8, sleep_s=20):
    """Poke the axon-tunneled devices until they respond. A crashed prior
    process can leave the remote exec unit wedged for a minute or two;
    the terminal recycles it on subsequent connection attempts."""
    import time

    import jax
    import jax.numpy as jnp

    for attempt in range(max_tries):
        try:
            devs = jax.devices()
            for d in devs[:1]:
                a = jax.device_put(jnp.ones((2, 2)), d)
                np.asarray(a)
            return True
        except Exception as exc:  # noqa: BLE001
            if attempt == max_tries - 1:
                raise
            print(f"device not ready (attempt {attempt + 1}): {exc}; retrying")
            time.sleep(sleep_s)
    return False


def run_spmd(in_maps, mode=MODE, **kwargs):
    nc = get_built(mode)
    wait_device_ready()
    try:
        return run_bass_kernel_spmd(
            nc, in_maps, core_ids=list(range(N_CORES)), **kwargs
        )
    except Exception as exc:  # noqa: BLE001
        print(f"run_bass_kernel_spmd failed ({exc}); retrying once after re-poke")
        wait_device_ready()
        return run_bass_kernel_spmd(
            nc, in_maps, core_ids=list(range(N_CORES)), **kwargs
        )


def kernel(x, W_experts, b_experts, W_router, b_router):
    in_maps = prep_inputs(x, W_experts, b_experts, W_router, b_router)
    res = run_spmd(in_maps)
    out = np.concatenate(
        [np.asarray(res.results[c]["out"]).astype(np.float32) for c in range(N_CORES)],
        axis=0,
    )
    return out.reshape(B, S, D)
